# revision 1
# baseline (speedup 1.0000x reference)
"""Circular-pad -> unfold(K=7,S=3) -> 896->64->896 MLP -> fold -> crop, on 8 NeuronCores.

Data-parallel: one batch element per core. The unfold/fold are never
materialized. Stage 1 is 7 PSUM-accumulated matmuls per tile whose rhs reads
the padded input directly with stride-3 column APs (no phase-split copies) and
whose stationary holds W1 twice (out rows 0:64 and 64:128), so the eviction
writes h into H2 rows 0:64 at its natural column and rows 64:128 shifted one
column right -- a [h[t]; h[t-1]] stack for free. Stage 2 then needs only 4
matmuls per tile instead of 7: each output stream (out col mod 3) packs its
2-3 fold taps into 128-deep contractions against H2. b2 rides as a
per-partition bias on the PSUM evictions (sum of each stream's taps), with a
2-column correction at the fold boundaries. H2, the stage-2 weights, and the
outputs are bf16 (rel tolerance 2e-2 >> bf16's ~4e-3): output HBM traffic
halves. Output is 3 phase-contiguous DRAM tensors; the host upcasts and
interleaves them (out[:, r::3] = o_r) during the gather.
"""

import ml_dtypes
import numpy as np

import concourse.bass as bass  # noqa: F401
import concourse.mybir as mybir
import concourse.tile as tile
from concourse import bacc
from concourse.bass_utils import run_bass_kernel_spmd
from concourse.tile import add_dep_helper

B, C, L = 8, 128, 16384
K, S, PAD, IC = 7, 3, 3, 64
LP = L + 2 * PAD          # 16390
P = (LP - K) // S + 1     # 5462
W = 512                   # max tile width (one PSUM bank of fp32)
# Variable tile widths: four 256-wide tiles ramp the pipeline (real PE work
# starts the moment the warm-up matmuls finish, keeping the HAM clock-gate
# at 2.4GHz), then full 512-wide tiles.
TILEW = [256] * 4 + [512] * 8 + [342]
TILE0 = [sum(TILEW[:j]) for j in range(len(TILEW))]
N1 = len(TILEW)           # 13 tiles
NCORES = 8
F32 = mybir.dt.float32
F32R = mybir.dt.float32r
BF16 = mybir.dt.bfloat16
HW2 = P + 1               # H2 columns: t in [0, 5462]
STRIDED_RHS = True        # bf16 cast pass + stride-3 rhs APs
NPH = [5464, 5463, 5463]  # phase buffer widths (fallback path)
# input chunk edges in x columns, aligned so chunk j delivers exactly what
# stage-1 tile j needs (xp cols up to 3*p0 + 3*w + 4)
XEDGES = [0] + [min(3 * p0 + 3 * w + 4 - PAD, L)
                for p0, w in zip(TILE0, TILEW)]

AOP = mybir.AluOpType
AF = mybir.ActivationFunctionType


def _ceil_div(a, b):
    return (a + b - 1) // b


def _body(tc, o0, o1, o2, x, w1d, w2p, b1d, b2s, b2f):
    nc = tc.nc

    with (
        tc.tile_pool(name="const", bufs=1) as cpool,
        tc.tile_pool(name="big", bufs=1) as bigpool,
        tc.tile_pool(name="stg", bufs=24) as stg,
        tc.tile_pool(name="ps1", bufs=2, space="PSUM") as ps1,
        tc.tile_pool(name="ps2", bufs=5, space="PSUM") as ps2,
    ):
        # --- constants (host-arranged) ---
        w1t = cpool.tile([C, K * 2 * IC],
                         BF16 if STRIDED_RHS else F32R)  # [c,128k+o]=W1[o,7c+k] dup@+64
        const_dmas = [nc.gpsimd.dma_start(out=w1t[:], in_=w1d)]
        w2t = cpool.tile([C, 4 * C], BF16)       # M0a | M1 | M2 | M0b stationaries
        const_dmas.append(nc.gpsimd.dma_start(out=w2t[:], in_=w2p))
        b1t = cpool.tile([C, 1], F32)            # b1 duplicated to both halves
        const_dmas.append(nc.gpsimd.dma_start(out=b1t[:], in_=b1d))
        b2st = cpool.tile([C, 3], F32)           # per-stream folded b2
        const_dmas.append(nc.gpsimd.dma_start(out=b2st[:], in_=b2s))
        b2ft = cpool.tile([C, 2], F32)           # boundary corrections
        const_dmas.append(nc.gpsimd.dma_start(out=b2ft[:], in_=b2f))

        # --- PE warm-up: fp32r dummy matmuls so HAM un-throttles early ---
        # (memset can't target fp32r; write the bits through an fp32 view)
        junk = cpool.tile([C, W], F32R)
        nc.vector.memset(junk[:].bitcast(F32), 0.0)
        psw = ps1.tile([C, W], F32, tag="ps1", name="psw")
        for _ in range(8):
            nc.tensor.matmul(psw[:, :], junk[:, 0:C], junk[:, :],
                             start=True, stop=True)
        # pull the activation table load into the ramp (off the h chain);
        # write to a scratch fp32 tile -- the verifier requires everything
        # feeding an fp32r matmul (junk) to be produced as rounded fp32r.
        scratch = cpool.tile([C, 2], F32)
        nc.scalar.activation(scratch[:], junk[:, 0:2].bitcast(F32),
                             AF.Relu, bias=0.0)

        # --- H2 edge columns: A[P]=0 (kills tap k=0 at u=P-1), B[0]=0 (k=6, u=0)
        h2 = bigpool.tile([C, HW2], BF16)
        nc.vector.memset(h2[0:IC, HW2 - 1:HW2], 0.0)
        nc.vector.memset(h2[IC:C, 0:1], 0.0)

        # --- padded input (circular), raw fp32 via sync HWDGE. Chunk 0
        # (exactly what stage-1 tile 0 needs) and the tiny const DMAs get HBM
        # priority: every other chunk waits for them so the first tile's
        # dependencies don't crawl behind the bulk-input storm on the rings.
        xf = bigpool.tile([C, LP], F32)
        nc.sync.dma_start(out=xf[:, 0:PAD], in_=x[:, L - PAD:L])
        nc.sync.dma_start(out=xf[:, PAD + L:LP], in_=x[:, 0:PAD])
        first = None
        chunk_dmas = []
        for a, b in zip(XEDGES[:-1], XEDGES[1:]):
            dma = nc.sync.dma_start(out=xf[:, PAD + a:PAD + b], in_=x[:, a:b])
            chunk_dmas.append(dma)
            if first is None:
                first = dma
            else:
                add_dep_helper(dma.ins, first.ins, sync=True,
                               reason="chunk0 priority")
                for cd in const_dmas:
                    add_dep_helper(dma.ins, cd.ins, sync=True,
                                   reason="const priority")

        if STRIDED_RHS:
            # One contiguous fp32->bf16 cast pass per chunk (alternating
            # engines); stage-1 matmuls then read xb with stride-3 column
            # APs at full rate (bf16 has no fp32r rounding constraint).
            xb = bigpool.tile([C, LP], BF16)

            def rhs_for(k, p0, w):
                a = 3 * p0 + k
                return xb[:, a: a + 3 * (w - 1) + 1: 3]

            cast_idx = [0]

            def phase_split(ca, cb):
                if cast_idx[0] % 2 == 0:
                    nc.vector.tensor_copy(out=xb[:, ca:cb], in_=xf[:, ca:cb])
                else:
                    nc.scalar.activation(xb[:, ca:cb], xf[:, ca:cb], AF.Copy)
                cast_idx[0] += 1
        else:
            xph = [bigpool.tile([C, NPH[r]], F32R, tag=f"xph{r}",
                                name=f"xph{r}") for r in range(3)]

            def phase_split(ca, cb):
                for r in range(3):
                    qa, qb = _ceil_div(ca - r, 3), _ceil_div(cb - r, 3)
                    src = xf[:, 3 * qa + r: 3 * (qb - 1) + r + 1: 3]
                    if r == 1:
                        nc.scalar.activation(xph[r][:, qa:qb], src, AF.Copy)
                    else:
                        nc.vector.tensor_copy(out=xph[r][:, qa:qb], in_=src)

            def rhs_for(k, p0, w):
                return xph[k % 3][:, k // 3 + p0: k // 3 + p0 + w]

        def stage1(i):
            p0, w = TILE0[i], TILEW[i]
            ps = ps1.tile([C, W], F32, tag="ps1")
            for k in range(K):
                nc.tensor.matmul(
                    ps[:, :w],
                    w1t[:, k * C:(k + 1) * C],
                    rhs_for(k, p0, w),
                    start=(k == 0),
                    stop=(k == K - 1),
                )
            # h into rows 0:64 (natural) and rows 64:128 (shifted +1 col)
            nc.scalar.activation(
                h2[0:IC, p0:p0 + w], ps[0:IC, :w],
                AF.Relu, bias=b1t[0:IC],
            )
            nc.vector.tensor_scalar(
                out=h2[IC:C, p0 + 1:p0 + 1 + w], in0=ps[IC:C, :w],
                scalar1=b1t[IC:C], scalar2=0.0, op0=AOP.add, op1=AOP.max,
            )

        def stage2(j):
            u0, w0 = TILE0[j], TILEW[j]   # stream-0 / matmul width (even)
            w12 = min(w0, P - 1 - u0)     # stream-1/2 eviction width
            rhs_hi = h2[:, u0 + 1:u0 + 1 + w0]
            ps_0 = ps2.tile([C, W], F32, tag="ps2")
            nc.tensor.matmul(ps_0[:, :w0], w2t[:, 0:C], rhs_hi,
                             start=True, stop=False)
            nc.tensor.matmul(ps_0[:, :w0], w2t[:, 3 * C:4 * C],
                             h2[:, u0:u0 + w0], start=False, stop=True)
            ps_1 = ps2.tile([C, W], F32, tag="ps2")
            nc.tensor.matmul(ps_1[:, :w0], w2t[:, C:2 * C], rhs_hi,
                             start=True, stop=True)
            ps_2 = ps2.tile([C, W], F32, tag="ps2")
            nc.tensor.matmul(ps_2[:, :w0], w2t[:, 2 * C:3 * C], rhs_hi,
                             start=True, stop=True)

            blk0 = stg.tile([C, W], BF16, tag="blk")
            nc.scalar.activation(blk0[:, :w0], ps_0[:, :w0],
                                 AF.Identity, bias=b2st[:, 0:1])
            if j == 0:
                nc.vector.tensor_scalar_sub(
                    blk0[:, 0:1], blk0[:, 0:1], b2ft[:, 0:1])
            if j == N1 - 1:
                nc.vector.tensor_scalar_sub(
                    blk0[:, w0 - 1:w0], blk0[:, w0 - 1:w0], b2ft[:, 1:2])
            blk1 = stg.tile([C, W], BF16, tag="blk")
            nc.vector.tensor_scalar_add(blk1[:, :w12], ps_1[:, :w12],
                                        b2st[:, 1:2])
            blk2 = stg.tile([C, W], BF16, tag="blk")
            nc.vector.tensor_scalar_add(blk2[:, :w12], ps_2[:, :w12],
                                        b2st[:, 2:3])
            # o0 on sync HWDGE and o1 on gpsimd SWDGE carry an input-priority
            # dep (they defer until the input stream is ahead); o2 issues
            # freely from the scalar HWDGE ring -- a dep there would stall
            # the next tile's activations behind it in the in-order queue.
            d0 = nc.sync.dma_start(out=o0[:, u0:u0 + w0], in_=blk0[:, :w0])
            d1 = nc.gpsimd.dma_start(out=o1[:, u0:u0 + w12], in_=blk1[:, :w12])
            nc.scalar.dma_start(out=o2[:, u0:u0 + w12], in_=blk2[:, :w12])
            dep = chunk_dmas[min(j + 2, len(chunk_dmas) - 1)]
            for dd in (d0, d1):
                add_dep_helper(dd.ins, dep.ins, sync=True,
                               reason="input priority")

        # Emit in dataflow order: chunk j's cast, then stage-1 tile j (which
        # it completes exactly), then the trailing stage-2 tile. The
        # per-engine sequencers execute in program order, so emitting casts
        # upfront would block every later eviction behind casts that wait on
        # not-yet-arrived chunks.
        edges = [0] + [PAD + a for a in XEDGES[1:-1]] + [LP]
        for j, (ca, cb) in enumerate(zip(edges[:-1], edges[1:])):
            phase_split(ca, cb)
            stage1(j)
            if j >= 1:
                stage2(j - 1)
        stage2(N1 - 1)


_CACHE = {}


def _build():
    if "nc" in _CACHE:
        return _CACHE["nc"]
    nc = bacc.Bacc("TRN2", target_bir_lowering=False, debug=False,
                   num_devices=NCORES)
    x = nc.dram_tensor("x", [C, L], F32, kind="ExternalInput").ap()
    w1d = nc.dram_tensor("w1d", [C, K * 2 * IC],
                         BF16 if STRIDED_RHS else F32, kind="ExternalInput").ap()
    w2p = nc.dram_tensor("w2p", [C, 4 * C], BF16, kind="ExternalInput").ap()
    b1d = nc.dram_tensor("b1d", [C, 1], F32, kind="ExternalInput").ap()
    b2s = nc.dram_tensor("b2s", [C, 3], F32, kind="ExternalInput").ap()
    b2f = nc.dram_tensor("b2f", [C, 2], F32, kind="ExternalInput").ap()
    o0 = nc.dram_tensor("o0", [C, P], BF16, kind="ExternalOutput").ap()
    o1 = nc.dram_tensor("o1", [C, P - 1], BF16, kind="ExternalOutput").ap()
    o2 = nc.dram_tensor("o2", [C, P - 1], BF16, kind="ExternalOutput").ap()
    with tile.TileContext(nc) as tc:
        _body(tc, o0, o1, o2, x, w1d, w2p, b1d, b2s, b2f)
    nc.compile()
    _CACHE["nc"] = nc
    return nc


def _prep_weights(W1, b1, W2, b2):
    # w1d[c, 128k + o] = w1d[c, 128k + 64 + o] = W1[o, 7c+k]
    w1blk = W1.reshape(IC, C, K).transpose(1, 2, 0)          # [c, k, o]
    w1d = np.concatenate([w1blk, w1blk], axis=2).reshape(C, K * 2 * IC)
    w1d = np.ascontiguousarray(
        w1d.astype(ml_dtypes.bfloat16) if STRIDED_RHS
        else w1d.astype(np.float32))
    # stage-2 stationaries [contraction, out_c]: rhs rows 0:64 = h[t] (A),
    # rows 64:128 = h[t-1] (B).
    W2r = W2.reshape(C, K, IC)                               # [c, k, o]
    m0a = np.concatenate([W2r[:, 0, :].T, W2r[:, 3, :].T], axis=0)
    m1 = np.concatenate([W2r[:, 1, :].T, W2r[:, 4, :].T], axis=0)
    m2 = np.concatenate([W2r[:, 2, :].T, W2r[:, 5, :].T], axis=0)
    m0b = np.concatenate([np.zeros((IC, C), np.float32), W2r[:, 6, :].T],
                         axis=0)
    w2p = np.ascontiguousarray(
        np.concatenate([m0a, m1, m2, m0b], axis=1).astype(ml_dtypes.bfloat16))
    b1d = np.ascontiguousarray(
        np.concatenate([b1, b1]).reshape(C, 1), dtype=np.float32)
    b2r = b2.reshape(C, K)
    b2s = np.ascontiguousarray(
        np.stack([b2r[:, 0] + b2r[:, 3] + b2r[:, 6],
                  b2r[:, 1] + b2r[:, 4],
                  b2r[:, 2] + b2r[:, 5]], axis=1), dtype=np.float32)
    b2f = np.ascontiguousarray(
        np.stack([b2r[:, 6], b2r[:, 0]], axis=1), dtype=np.float32)
    return w1d, w2p, b1d, b2s, b2f


def kernel(x, W1, b1, W2, b2, _trace=False):
    nc = _build()
    w1d, w2p, b1d, b2s, b2f = _prep_weights(
        np.asarray(W1, np.float32), np.asarray(b1, np.float32),
        np.asarray(W2, np.float32), np.asarray(b2, np.float32))
    x = np.asarray(x, np.float32)
    in_maps = [
        {"x": np.ascontiguousarray(x[i]), "w1d": w1d, "w2p": w2p,
         "b1d": b1d, "b2s": b2s, "b2f": b2f}
        for i in range(NCORES)
    ]
    res = run_bass_kernel_spmd(nc, in_maps, core_ids=list(range(NCORES)),
                               trace=_trace)
    out = np.empty((NCORES, C, L), np.float32)
    for i, r in enumerate(res.results):
        out[i, :, 0::3] = np.asarray(r["o0"]).astype(np.float32)
        out[i, :, 1::3] = np.asarray(r["o1"]).astype(np.float32)
        out[i, :, 2::3] = np.asarray(r["o2"]).astype(np.float32)
    if _trace:
        kernel.last_results = res
    return out

